# revision 6
# baseline (speedup 1.0000x reference)
"""Conv2d 3x3 (stride 1, pad 1) Bass kernel for TRN2, 8-core SPMD.

Problem: x [32, 64, 56, 56] f32, filters [128, 64, 3, 3] f32
         -> out [32, 128, 56, 56] f32.

Sharding: data-parallel over batch, 4 images per core.

v3 design:
  - bf16 on-chip: x, weights, staged output bf16 (f32 PSUM accumulate);
    host converts the bf16 result back to f32.  rel err ~2.9e-3.
  - Conv = 9 shifted K=64 matmul taps accumulated in PSUM; two images
    run concurrently as 64-row PE tiles (rows 0/64).  PE moving stream
    (128 rows x 1 col/cycle) is the roofline: 126 pair-slots x ~195 ns.
  - bf16 weight loads use the HW fast-weight-load path (~147 ns) and
    hide under the 187 ns matmuls, so plain 1:1 LDW with row-block-
    outer/tap-inner order (smooth chunked-DMA consumption) is optimal.
  - Weights then x stream on the sync queue in consumption order;
    output DMAs trigger from the gpsimd queue so they never serialize
    behind input descriptor generation.
  - Copies: DVE takes row-blocks 0-3, ACT takes 4-6, so each output
    DMA chunk depends on a single engine semaphore (fewer event-
    semaphore splits -> shorter kernel-exit tail).
"""

import sys

sys.path.insert(0, "/opt/trn_rl_repo")

import numpy as np
import ml_dtypes

B, C, H, W = 32, 64, 56, 56
OC = 128
KH = KW = 3
NCORES = 8
BPC = B // NCORES          # images per core (4)
HP, WP = H + 2, W + 2      # padded 58x58
IMG = HP * WP              # 3364 padded image size per channel
STRIP = 2                  # images per partition-strip
L = STRIP * IMG            # free-dim length of the x tensor (6728)
RB = 8                     # output rows per tile
NT = RB * W                # matmul free size (448)
NRB = H // RB              # row blocks per image (7)
OUT_IMG = H * W            # 3136
G0R = 4                    # row-blocks drained in the first output chunk

_cache = {}


def _build():
    import concourse.mybir as mybir
    import concourse.tile as tile
    from concourse import bacc

    nc = bacc.Bacc("TRN2", target_bir_lowering=False, debug=False,
                   num_devices=NCORES)
    x_ext = nc.declare_dram_parameter("x2", [2 * C, L], mybir.dt.bfloat16,
                                      isOutput=False)
    w_ext = nc.declare_dram_parameter("wt", [2 * C, KH * KW * OC],
                                      mybir.dt.bfloat16, isOutput=False)
    y_ext = nc.declare_dram_parameter("y", [BPC, OC, OUT_IMG],
                                      mybir.dt.bfloat16, isOutput=True)

    with tile.TileContext(nc) as tc:
        with (
            tc.tile_pool(name="xp", bufs=1) as xp,
            tc.tile_pool(name="wp", bufs=1) as wp,
            tc.tile_pool(name="ps", bufs=8, space="PSUM") as ps,
            tc.tile_pool(name="op", bufs=2) as op,
        ):
            w_t = wp.tile([2 * C, KH * KW * OC], mybir.dt.bfloat16)
            x_t = xp.tile([2 * C, L], mybir.dt.bfloat16)
            # weights on the scalar ring (tap 0 first), x alone on the
            # sync ring in consumption order -- the two rings run in
            # parallel so the first matmuls are gated only by ~1 chunk
            nc.scalar.dma_start(w_t[:, 0:OC], w_ext.ap()[:, 0:OC])
            nc.scalar.dma_start(w_t[:, OC:KH * KW * OC],
                                w_ext.ap()[:, OC:KH * KW * OC])
            for q in range(STRIP):
                base = q * IMG
                if q == 0:
                    bounds = [0, 10 * WP] + [(10 + 8 * i) * WP
                                             for i in range(1, NRB - 1)]
                    bounds.append(IMG)
                else:
                    bounds = [0, 34 * WP, IMG]
                for i in range(len(bounds) - 1):
                    lo, hi = bounds[i], bounds[i + 1]
                    nc.sync.dma_start(x_t[:, base + lo:base + hi],
                                      x_ext.ap()[:, base + lo:base + hi])
            x4 = x_t[:].rearrange("p (i r w) -> p i r w", i=STRIP, w=WP)

            # PE warm-up on a zeroed scratch tile (no DMA deps): ramps the
            # HAM clock gate while the DMA queues spin up (~9us)
            wsrc = wp.tile([2 * C, 512], mybir.dt.bfloat16, tag="warmsrc")
            nc.gpsimd.memset(wsrc[:], 0.0)
            for wn in range(8):
                h = wn % 2
                warm = ps.tile([OC, NT], mybir.dt.float32, tag="ps",
                               name="pst")
                nc.tensor.matmul(warm[:], wsrc[h * C:h * C + C, 0:OC],
                                 wsrc[h * C:h * C + C, 0:NT],
                                 start=True, stop=True,
                                 skip_group_check=True)

            for q in range(STRIP):
                o_tiles = [
                    op.tile([OC, OUT_IMG], mybir.dt.bfloat16, tag=f"o{h}",
                            name=f"ot{h}") for h in (0, 1)
                ]
                for r in range(NRB):
                    pts = [ps.tile([OC, NT], mybir.dt.float32, tag="ps",
                                   name="pst") for _ in (0, 1)]
                    for tap in range(KH * KW):
                        kh, kw = divmod(tap, KW)
                        h0 = r * RB + kh
                        wsl = slice(tap * OC, (tap + 1) * OC)
                        for h in (0, 1):
                            rhs = x4[h * C:h * C + C, q, h0:h0 + RB,
                                     kw:kw + W]
                            nc.tensor.matmul(
                                pts[h][:], w_t[h * C:h * C + C, wsl], rhs,
                                start=(tap == 0), stop=(tap == KH * KW - 1))
                    lo, hi = r * NT, (r + 1) * NT
                    for h in (0, 1):
                        dst = o_tiles[h][:, lo:hi]
                        if r % 2 == 0:
                            nc.vector.tensor_copy(dst, pts[h][:])
                        else:
                            nc.scalar.copy(dst, pts[h][:])
                        # per-row-block drain: single-copy dependency,
                        # flows out right behind the copy
                        nc.gpsimd.dma_start(
                            y_ext.ap()[q + h * STRIP, :, lo:hi],
                            o_tiles[h][:, lo:hi])

    nc.compile()
    return nc


def _prep_inputs(x, filters):
    """Host-side pad + bf16 quantize: returns per-core in_maps."""
    xpad = np.zeros((B, C, HP, WP), dtype=np.float32)
    xpad[:, :, 1:1 + H, 1:1 + W] = x
    xpad16 = xpad.astype(ml_dtypes.bfloat16)
    wt = np.empty((2 * C, KH * KW * OC), dtype=ml_dtypes.bfloat16)
    for tap in range(KH * KW):
        kh, kw = divmod(tap, KW)
        wtap = filters[:, :, kh, kw].T.astype(ml_dtypes.bfloat16)  # [C, OC]
        wt[0:C, tap * OC:(tap + 1) * OC] = wtap
        wt[C:2 * C, tap * OC:(tap + 1) * OC] = wtap
    in_maps = []
    for c in range(NCORES):
        xc = xpad16[c * BPC:(c + 1) * BPC]                # [4, C, HP, WP]
        lower = xc[0:2].transpose(1, 0, 2, 3).reshape(C, L)
        upper = xc[2:4].transpose(1, 0, 2, 3).reshape(C, L)
        x2 = np.ascontiguousarray(np.concatenate([lower, upper], axis=0))
        in_maps.append({"x2": x2, "wt": wt})
    return in_maps


def kernel(x, filters):
    from concourse.bass_utils import run_bass_kernel_spmd

    x = np.asarray(x, dtype=np.float32)
    filters = np.asarray(filters, dtype=np.float32)
    if "nc" not in _cache:
        _cache["nc"] = _build()
    nc = _cache["nc"]
    in_maps = _prep_inputs(x, filters)
    res = run_bass_kernel_spmd(nc, in_maps, core_ids=list(range(NCORES)))
    out = np.empty((B, OC, H, W), dtype=np.float32)
    for c in range(NCORES):
        y = np.asarray(res.results[c]["y"], dtype=np.float32)  # [4,OC,3136]
        out[c * BPC:(c + 1) * BPC] = y.reshape(BPC, OC, H, W)
    return out


if __name__ == "__main__":
    rng = np.random.default_rng(0)
    x = rng.standard_normal((B, C, H, W), dtype=np.float32)
    f = rng.standard_normal((OC, C, KH, KW), dtype=np.float32)
    out = kernel(x, f)
    print("out", out.shape, out.dtype, float(np.abs(out).mean()))


# revision 7
# speedup vs baseline: 1.0975x; 1.0975x over previous
"""Conv2d 3x3 (stride 1, pad 1) Bass kernel for TRN2, 8-core SPMD.

Problem: x [32, 64, 56, 56] f32, filters [128, 64, 3, 3] f32
         -> out [32, 128, 56, 56] f32.

Sharding: data-parallel over batch, 4 images per core.

v3 design:
  - bf16 on-chip: x, weights, staged output bf16 (f32 PSUM accumulate);
    host converts the bf16 result back to f32.  rel err ~2.9e-3.
  - Conv = 9 shifted K=64 matmul taps accumulated in PSUM; two images
    run concurrently as 64-row PE tiles (rows 0/64).  PE moving stream
    (128 rows x 1 col/cycle) is the roofline: 126 pair-slots x ~195 ns.
  - bf16 weight loads use the HW fast-weight-load path (~147 ns) and
    hide under the 187 ns matmuls, so plain 1:1 LDW with row-block-
    outer/tap-inner order (smooth chunked-DMA consumption) is optimal.
  - Weights then x stream on the sync queue in consumption order;
    output DMAs trigger from the gpsimd queue so they never serialize
    behind input descriptor generation.
  - Copies: DVE takes row-blocks 0-3, ACT takes 4-6, so each output
    DMA chunk depends on a single engine semaphore (fewer event-
    semaphore splits -> shorter kernel-exit tail).
"""

import sys

sys.path.insert(0, "/opt/trn_rl_repo")

import numpy as np
import ml_dtypes

B, C, H, W = 32, 64, 56, 56
OC = 128
KH = KW = 3
NCORES = 8
BPC = B // NCORES          # images per core (4)
HP, WP = H + 2, W + 2      # padded 58x58
IMG = HP * WP              # 3364 padded image size per channel
STRIP = 2                  # images per partition-strip
L = STRIP * IMG            # free-dim length of the x tensor (6728)
RB = 8                     # output rows per tile
NT = RB * W                # matmul free size (448)
NRB = H // RB              # row blocks per image (7)
OUT_IMG = H * W            # 3136
G0R = 4                    # row-blocks drained in the first output chunk

_cache = {}


def _build():
    import concourse.mybir as mybir
    import concourse.tile as tile
    from concourse import bacc

    nc = bacc.Bacc("TRN2", target_bir_lowering=False, debug=False,
                   num_devices=NCORES)
    x_ext = nc.declare_dram_parameter("x2", [2 * C, L], mybir.dt.bfloat16,
                                      isOutput=False)
    w_ext = nc.declare_dram_parameter("wt", [2 * C, KH * KW * OC],
                                      mybir.dt.bfloat16, isOutput=False)
    y_ext = nc.declare_dram_parameter("y", [BPC, OC, OUT_IMG],
                                      mybir.dt.bfloat16, isOutput=True)

    with tile.TileContext(nc) as tc:
        with (
            tc.tile_pool(name="xp", bufs=1) as xp,
            tc.tile_pool(name="wp", bufs=1) as wp,
            tc.tile_pool(name="ps", bufs=8, space="PSUM") as ps,
            tc.tile_pool(name="op", bufs=2) as op,
        ):
            w_t = wp.tile([2 * C, KH * KW * OC], mybir.dt.bfloat16)
            x_t = xp.tile([2 * C, L], mybir.dt.bfloat16)
            # single ordered input ring (sync), sequenced so that the
            # first row-blocks' weights and rows land just ahead of
            # their consumption: taps 0-2, image-0 rows 0-17, taps 3-8,
            # then the remaining rows
            nc.sync.dma_start(w_t[:, 0:3 * OC], w_ext.ap()[:, 0:3 * OC])
            nc.sync.dma_start(x_t[:, 0:10 * WP], x_ext.ap()[:, 0:10 * WP])
            nc.sync.dma_start(x_t[:, 10 * WP:18 * WP],
                              x_ext.ap()[:, 10 * WP:18 * WP])
            nc.sync.dma_start(w_t[:, 3 * OC:KH * KW * OC],
                              w_ext.ap()[:, 3 * OC:KH * KW * OC])
            for q in range(STRIP):
                base = q * IMG
                if q == 0:
                    bounds = [18 * WP] + [(10 + 8 * i) * WP
                                          for i in range(2, NRB - 1)]
                    bounds.append(IMG)
                else:
                    bounds = [0, 34 * WP, IMG]
                for i in range(len(bounds) - 1):
                    lo, hi = bounds[i], bounds[i + 1]
                    nc.sync.dma_start(x_t[:, base + lo:base + hi],
                                      x_ext.ap()[:, base + lo:base + hi])
            x4 = x_t[:].rearrange("p (i r w) -> p i r w", i=STRIP, w=WP)

            # PE warm-up on a zeroed scratch tile (no DMA deps): ramps the
            # HAM clock gate while the DMA queues spin up (~9us)
            wsrc = wp.tile([2 * C, 512], mybir.dt.bfloat16, tag="warmsrc")
            nc.vector.memset(wsrc[:], 0.0)
            for wn in range(8):
                h = wn % 2
                warm = ps.tile([OC, NT], mybir.dt.float32, tag="ps",
                               name="pst")
                nc.tensor.matmul(warm[:], wsrc[h * C:h * C + C, 0:OC],
                                 wsrc[h * C:h * C + C, 0:NT],
                                 start=True, stop=True,
                                 skip_group_check=True)

            for q in range(STRIP):
                o_tiles = [
                    op.tile([OC, OUT_IMG], mybir.dt.bfloat16, tag=f"o{h}",
                            name=f"ot{h}") for h in (0, 1)
                ]
                for r in range(NRB):
                    pts = [ps.tile([OC, NT], mybir.dt.float32, tag="ps",
                                   name="pst") for _ in (0, 1)]
                    for tap in range(KH * KW):
                        kh, kw = divmod(tap, KW)
                        h0 = r * RB + kh
                        wsl = slice(tap * OC, (tap + 1) * OC)
                        for h in (0, 1):
                            rhs = x4[h * C:h * C + C, q, h0:h0 + RB,
                                     kw:kw + W]
                            nc.tensor.matmul(
                                pts[h][:], w_t[h * C:h * C + C, wsl], rhs,
                                start=(tap == 0), stop=(tap == KH * KW - 1))
                    lo, hi = r * NT, (r + 1) * NT
                    for h in (0, 1):
                        dst = o_tiles[h][:, lo:hi]
                        if r % 2 == 0:
                            nc.vector.tensor_copy(dst, pts[h][:])
                        else:
                            nc.scalar.copy(dst, pts[h][:])
                    if r % 2 == 1 or r == NRB - 1:
                        clo = (r - 1) * NT if r % 2 == 1 else r * NT
                        eng = nc.gpsimd if q == 0 else nc.scalar
                        for h in (0, 1):
                            eng.dma_start(
                                y_ext.ap()[q + h * STRIP, :, clo:hi],
                                o_tiles[h][:, clo:hi])

    nc.compile()
    return nc


def _prep_inputs(x, filters):
    """Host-side pad + bf16 quantize: returns per-core in_maps."""
    xpad = np.zeros((B, C, HP, WP), dtype=np.float32)
    xpad[:, :, 1:1 + H, 1:1 + W] = x
    xpad16 = xpad.astype(ml_dtypes.bfloat16)
    wt = np.empty((2 * C, KH * KW * OC), dtype=ml_dtypes.bfloat16)
    for tap in range(KH * KW):
        kh, kw = divmod(tap, KW)
        wtap = filters[:, :, kh, kw].T.astype(ml_dtypes.bfloat16)  # [C, OC]
        wt[0:C, tap * OC:(tap + 1) * OC] = wtap
        wt[C:2 * C, tap * OC:(tap + 1) * OC] = wtap
    in_maps = []
    for c in range(NCORES):
        xc = xpad16[c * BPC:(c + 1) * BPC]                # [4, C, HP, WP]
        lower = xc[0:2].transpose(1, 0, 2, 3).reshape(C, L)
        upper = xc[2:4].transpose(1, 0, 2, 3).reshape(C, L)
        x2 = np.ascontiguousarray(np.concatenate([lower, upper], axis=0))
        in_maps.append({"x2": x2, "wt": wt})
    return in_maps


def kernel(x, filters):
    from concourse.bass_utils import run_bass_kernel_spmd

    x = np.asarray(x, dtype=np.float32)
    filters = np.asarray(filters, dtype=np.float32)
    if "nc" not in _cache:
        _cache["nc"] = _build()
    nc = _cache["nc"]
    in_maps = _prep_inputs(x, filters)
    res = run_bass_kernel_spmd(nc, in_maps, core_ids=list(range(NCORES)))
    out = np.empty((B, OC, H, W), dtype=np.float32)
    for c in range(NCORES):
        y = np.asarray(res.results[c]["y"], dtype=np.float32)  # [4,OC,3136]
        out[c * BPC:(c + 1) * BPC] = y.reshape(BPC, OC, H, W)
    return out


if __name__ == "__main__":
    rng = np.random.default_rng(0)
    x = rng.standard_normal((B, C, H, W), dtype=np.float32)
    f = rng.standard_normal((OC, C, KH, KW), dtype=np.float32)
    out = kernel(x, f)
    print("out", out.shape, out.dtype, float(np.abs(out).mean()))
